# revision 39
# baseline (speedup 1.0000x reference)
"""DGCNN + voxel-graph forward on 8 Trainium2 NeuronCores (Bass/Tile).

Sharding: data-parallel over the 64 point clouds for the point stage
(8 clouds/core); stage-1 per-cloud results are AllGathered on-device and the
tiny voxel-graph stage runs replicated on every core; host takes core 0's
output.

Key algorithmic mappings (validated vs the reference in fp32 numpy):
  - KNN ranking key s[n,m] = 2*x_n.x_m - ||x_m||^2 (row-constant term of the
    true squared distance dropped; ordering per row unchanged). Computed as
    ONE PE matmul per 128-row tile with an augmented contraction
    [ones; x] . [-xx; 2x].
  - top-16 per row = 2 rounds of the DVE max8/max_index/match_replace unit.
  - edge conv max_k lrelu(W.[nbr-ctr, ctr]) == lrelu(max_k(WL.x_m) + (WR-WL).x_n)
    (lrelu monotone, max over gathered u columns only).
  - neighbor gather via SWDGE dma_gather from a DRAM scratch of point-major
    u rows, 8x1024-idx calls round-robined over 4 SWDGE queues
    (single_packet=False); gathered rows land at (partition n%128, free
    slot 4k+t), so max over k is a strided DVE reduce. With 4 queues each
    queue's ucode reads the idx table from one fixed 16-partition block;
    the maskh/p8h fold replicates the table into all 8 blocks, so the same
    fold serves any queue assignment.
  - BN folded into per-channel scale/bias applied by the ACT engine's
    Prelu (alpha=0.2) activation, which also yields the mean-pool via
    accum_out.
"""

import numpy as np
from contextlib import ExitStack

import concourse.bass as bass
import concourse.bacc as bacc
import concourse.mybir as mybir
import concourse.tile as tile
from concourse import bass_utils

dt = mybir.dt
F32, F16, U16, I16 = dt.float32, dt.float16, dt.uint16, dt.int16
BF16 = dt.bfloat16
AF = mybir.ActivationFunctionType
AX = mybir.AxisListType
ALU = mybir.AluOpType

NCORES = 8
P = 512          # points per cloud
K = 16
VK = 8
V = 32           # voxels per outer cloud
CLS = 40
EPS = 1e-5
NEG_IMM = -3.0e38

# (C_in, O_out, O_pad, gather dtype, K_aug, ones_pos) per edge layer.
# O_pad*dtsize must be a multiple of 256B (dma_gather elem restriction).
# The KNN matmul contracts over [x (rows 0..C-1); pad; ones (row ones_pos)]
# against [2x; pad; -xx]; ones_pos must be a 32-aligned partition base.
LAYERS = [(9, 32, 64, F32, 33, 32), (32, 32, 64, F32, 33, 32),
          (32, 64, 64, F32, 33, 32), (64, 128, 128, F16, 65, 64)]
CLOUDS_PER_CORE = 8

_CACHE = {}


def _prep_host(inputs):
    """Host-side weight transforms (fp32, deterministic)."""
    g = lambda k: np.asarray(inputs[k], np.float32)
    d = {}
    d["ident"] = np.eye(128, dtype=np.float32)
    # fold helpers: idxs16[q, col] = idxU[16*(col%8)+q, c(col)] via one-hot mm
    maskh = np.zeros((128, 512), np.float16)
    for p in range(128):
        maskh[p, np.arange(512) % 8 == p // 16] = 1.0
    d["maskh"] = maskh
    # replicated one-hot: out row p' picks source partition with p%16 == p'%16
    # (dma_gather ucode: each of the 8 Q7 cores reads its own 16-part block)
    p8h = (np.arange(128)[:, None] % 16 == np.arange(128)[None, :] % 16)
    d["p8h"] = p8h.astype(np.float16)
    for li, wname in enumerate(["W1", "W2", "W3", "W4"]):
        W = g(wname)
        C = W.shape[1] // 2
        K_aug = LAYERS[li][4]
        wl = W[:, :C].T                                            # (C, O)
        wd = (W[:, C:] - W[:, :C]).T
        wluv = np.zeros((K_aug, 2 * wl.shape[1]), np.float32)
        wluv[0:C, :] = np.concatenate([wl, wd], axis=1)
        d[f"wluv{li}"] = wluv

    def bnf(gk, bk, mk, vk):
        s = g(gk) / np.sqrt(g(vk) + EPS)
        b = g(bk) - g(mk) * s
        return s.astype(np.float32), b.astype(np.float32)

    def cvec(x, ntile):  # (C,) -> (128, ntile) [partition, mtile]
        return np.ascontiguousarray(x.reshape(ntile, 128).T)

    s5, b5 = bnf("g5", "b5", "m5", "v5")
    d["s5"] = cvec(s5, 8); d["b5"] = cvec(b5, 8)
    import ml_dtypes
    d["w5t"] = np.ascontiguousarray(g("W5").T).astype(ml_dtypes.bfloat16)
    # MLP: BN scale folded into weight columns, biases as ones-row matmuls
    s6, b6 = bnf("g6", "b6", "m6", "v6")
    lin1 = g("lin1W").T.copy()                                      # (2048, 1024)
    lin1[1024:, :] *= 1.0 / 512.0                                   # mean pool fold
    lin1 *= s6[None, :]
    d["lin1wt"] = lin1
    d["b6row"] = b6.reshape(1, 1024)
    d["s6"] = cvec(s6, 8); d["b6"] = cvec(b6, 8)
    s7, b7 = bnf("g7", "b7", "m7", "v7")
    b7 = b7 + g("lin2b") * s7
    d["lin2wt"] = np.ascontiguousarray(g("lin2W").T) * s7[None, :]  # (1024, 1024)
    d["b7row"] = b7.reshape(1, 1024)
    d["s7"] = cvec(s7, 8); d["b7"] = cvec(b7, 8)
    d["lin3wt"] = np.ascontiguousarray(g("lin3W").T)                # (1024, 40)
    d["lin3b"] = g("lin3b").reshape(40, 1)
    d["lin3brow"] = g("lin3b").reshape(1, 40)

    W6 = g("W6")
    d["w6l"] = np.ascontiguousarray(W6[:, :40].T)                   # (40, 512)
    d["w6d"] = np.ascontiguousarray((W6[:, 40:] - W6[:, :40]).T)
    s6c, b6c = bnf("g6c", "b6c", "m6c", "v6c")
    d["s6c"] = cvec(s6c, 4); d["b6c"] = cvec(b6c, 4)
    W7 = g("W7")
    d["w7l"] = np.ascontiguousarray(W7[:, :512].T)                  # (512, 256)
    d["w7d"] = np.ascontiguousarray((W7[:, 512:] - W7[:, :512]).T)
    s7c, b7c = bnf("g7c", "b7c", "m7c", "v7c")
    d["s7c"] = cvec(s7c, 2); d["b7c"] = cvec(b7c, 2)
    d["w8t"] = np.ascontiguousarray(g("W8").T)                      # (768, 1024)
    s8c, b8c = bnf("g8c", "b8c", "m8c", "v8c")
    d["s8c"] = cvec(s8c, 8); d["b8c"] = cvec(b8c, 8)
    lin4 = g("lin4W").T.copy()                                      # (2048, 256)
    lin4[1024:, :] *= 1.0 / float(V)
    d["lin4wt"] = lin4
    d["lin5wt"] = np.ascontiguousarray(g("lin5W").T)                # (256, 40)
    d["lin5b"] = g("lin5b").reshape(40, 1)
    return d


NQ = 4  # SWDGE queues; gathers round-robin across them


def _build():
    nc = bacc.Bacc("TRN2", target_bir_lowering=False, debug=False,
                   num_devices=NCORES, num_swdge_queues=NQ)

    inp = nc.dram_tensor("inp", [CLOUDS_PER_CORE, P, 9], F32, kind="ExternalInput")
    din = {}
    for name, shape in [
        ("ident", [128, 128]), ("lin1wt", [2048, 1024]),
        ("lin2wt", [1024, 1024]), ("lin3wt", [1024, 40]), ("lin3b", [40, 1]),
        ("s5", [128, 8]), ("b5", [128, 8]), ("s6", [128, 8]), ("b6", [128, 8]),
        ("s7", [128, 8]), ("b7", [128, 8]),
        ("w6l", [40, 512]), ("w6d", [40, 512]), ("s6c", [128, 4]), ("b6c", [128, 4]),
        ("w7l", [512, 256]), ("w7d", [512, 256]), ("s7c", [128, 2]), ("b7c", [128, 2]),
        ("w8t", [768, 1024]), ("s8c", [128, 8]), ("b8c", [128, 8]),
        ("lin4wt", [2048, 256]), ("lin5wt", [256, 40]), ("lin5b", [40, 1]),
        ("b6row", [1, 1024]), ("b7row", [1, 1024]), ("lin3brow", [1, 40]),
    ]:
        din[name] = nc.dram_tensor(name, shape, F32, kind="ExternalInput")
    din["w5t"] = nc.dram_tensor("w5t", [256, 1024], BF16, kind="ExternalInput")
    din["maskh"] = nc.dram_tensor("maskh", [128, 512], F16, kind="ExternalInput")
    din["p8h"] = nc.dram_tensor("p8h", [128, 128], F16, kind="ExternalInput")
    for li, (C, O, _, _, K_aug, _) in enumerate(LAYERS):
        din[f"wluv{li}"] = nc.dram_tensor(f"wluv{li}", [K_aug, 2 * O], F32,
                                          kind="ExternalInput")

    out_d = nc.dram_tensor("out", [2, CLS], F32, kind="ExternalOutput")
    cc_out = nc.dram_tensor("cc_out", [64, CLS], F32, addr_space="Shared")

    with tile.TileContext(nc, num_cores=NCORES) as tc:
        with ExitStack() as pctx:
            pers = pctx.enter_context(tc.tile_pool(name="pers", bufs=1))
            dram = pctx.enter_context(tc.tile_pool(name="dram", bufs=2, space="DRAM"))

            ident = pers.tile([128, 128], F32, tag="ident")
            nc.sync.dma_start(ident, din["ident"][:, :])
            ones_col = pers.tile([128, 1], F32, tag="ones")
            nc.vector.memset(ones_col, 1.0)
            ones_row = pers.tile([1, 512], F32, tag="ones_row")
            nc.vector.memset(ones_row, 1.0)
            maskh_sb = pers.tile([128, 512], F16, tag="maskh")
            nc.sync.dma_start(maskh_sb, din["maskh"][:, :])
            p8h_sb = pers.tile([128, 128], F16, tag="p8h")
            nc.sync.dma_start(p8h_sb, din["p8h"][:, :])

            wluv_sb = []
            for li, (C, O, _, _, K_aug, _) in enumerate(LAYERS):
                t1 = pers.tile([K_aug, 2 * O], F32, tag=f"wluv{li}")
                nc.sync.dma_start(t1, din[f"wluv{li}"][:, :])
                wluv_sb.append(t1)

            w5g = []
            for gi, (r0, r1) in enumerate([(0, 128), (128, 256)]):
                t = pers.tile([r1 - r0, 1024], BF16, tag=f"w5g{gi}")
                nc.sync.dma_start(t, din["w5t"][r0:r1, :])
                w5g.append(t)

            def vec_tile(name, n):
                t = pers.tile([128, n], F32, tag=name)
                nc.sync.dma_start(t, din[name][:, :])
                return t
            s5 = vec_tile("s5", 8); b5 = vec_tile("b5", 8)
            s6 = vec_tile("s6", 8); b6 = vec_tile("b6", 8)
            s7 = vec_tile("s7", 8); b7 = vec_tile("b7", 8)
            s6c = vec_tile("s6c", 4); b6c = vec_tile("b6c", 4)
            s7c = vec_tile("s7c", 2); b7c = vec_tile("b7c", 2)
            s8c = vec_tile("s8c", 8); b8c = vec_tile("b8c", 8)
            lin3b = pers.tile([40, 1], F32, tag="lin3b")
            nc.sync.dma_start(lin3b, din["lin3b"][:, :])
            lin5b = pers.tile([40, 1], F32, tag="lin5b")
            nc.sync.dma_start(lin5b, din["lin5b"][:, :])
            lin3w = []
            for kk in range(8):
                t = pers.tile([128, 40], F32, tag=f"lin3w{kk}")
                nc.sync.dma_start(t, din["lin3wt"][128 * kk:128 * (kk + 1), :])
                lin3w.append(t)

            # pooled stats across the core's 8 clouds: [part, mtile, cloud]
            PM = pers.tile([128, 8, 8], F32, tag="PM")
            PS = pers.tile([128, 8, 8], F32, tag="PS")
            # stage-2 pooled: [part, mtile, b]
            PM2 = pers.tile([128, 8, 2], F32, tag="PM2")
            PS2 = pers.tile([128, 8, 2], F32, tag="PS2")

            cc_in = dram.tile([CLOUDS_PER_CORE, CLS], F32, tag="cc_in", bufs=1)

            # ============ stage 1: DGCNN, layer-major across clouds ============
            # All 8 clouds advance through each edge layer together so the 8
            # clouds' gathers queue back-to-back on the (bottleneck) GpSimd
            # engine while PE/DVE work for other clouds runs underneath.
            with ExitStack() as s1ctx:
                sbA = s1ctx.enter_context(tc.tile_pool(name="s1A", bufs=18))
                sb = s1ctx.enter_context(tc.tile_pool(name="s1", bufs=3))
                sbg = s1ctx.enter_context(tc.tile_pool(name="s1g", bufs=3))
                ps = s1ctx.enter_context(tc.tile_pool(name="s1ps", bufs=2, space="PSUM"))

                # ---- load input clouds; A1 = [ones; x] (10, 512) ----
                A_cur = []
                for cl in range(CLOUDS_PER_CORE):
                    A1 = sbA.tile([33, 512], F32, tag="A", name=f"A0_{cl}")
                    xfm_ps = ps.tile([9, 512], F32, tag="xx", bufs=1)
                    for t in range(4):
                        xpm = sb.tile([128, 9], F32, tag="xin")
                        nc.sync.dma_start(xpm, inp[cl, 128 * t:128 * (t + 1), :])
                        nc.tensor.transpose(xfm_ps[:, 128 * t:128 * (t + 1)], xpm, ident)
                    nc.vector.memset(A1[0:33, :], 0.0)
                    nc.scalar.copy(out=A1[0:9, :], in_=xfm_ps)
                    nc.vector.memset(A1[32:33, :], 1.0)
                    A_cur.append(A1)

                featd = [[None] * 3 for _ in range(CLOUDS_PER_CORE)]
                for li, (C, O, OP, GDT, K_aug, ONE) in enumerate(LAYERS):
                    A_nxt = []
                    for cl in range(CLOUDS_PER_CORE):
                        A = A_cur[cl]
                        X = A[0:C, :]
                        # ---- xx row ----
                        sq = sb.tile([C, 512], F32, tag="sq")
                        nc.vector.tensor_mul(sq, X, X)
                        xx_ps = ps.tile([1, 512], F32, tag="xx", bufs=1)
                        nc.tensor.matmul(xx_ps, ones_col[0:C, :], sq,
                                         start=True, stop=True)
                        # ---- B = [2x; pad; -xx] (augmented) ----
                        B = sb.tile([K_aug, 512], F32, tag="B")
                        if ONE != C:
                            nc.vector.memset(B[0:K_aug, :], 0.0)
                        nc.scalar.activation(B[0:C, :], X, AF.Identity,
                                             scale=2.0)
                        nc.scalar.activation(B[ONE:ONE + 1, :], xx_ps,
                                             AF.Identity, scale=-1.0)

                        # ---- ranking key + top-16 per 128-row tile ----
                        idxU = sb.tile([128, 64], U16, tag="idxU")
                        for t in range(4):
                            s_ps = ps.tile([128, 512], F32, tag="s_ps")
                            nc.tensor.matmul(s_ps,
                                             A[0:K_aug, 128 * t:128 * (t + 1)],
                                             B, start=True, stop=True)
                            v8 = sb.tile([128, 16], F32, tag="v8")
                            nc.vector.max(out=v8[:, 0:8], in_=s_ps)
                            nc.vector.max_index(out=idxU[:, 16 * t:16 * t + 8],
                                                in_max=v8[:, 0:8], in_values=s_ps)
                            sR = sb.tile([128, 512], F32, tag="sR", bufs=3, name="sR")
                            nc.vector.match_replace(out=sR, in_to_replace=v8[:, 0:8],
                                                    in_values=s_ps, imm_value=NEG_IMM)
                            nc.vector.max(out=v8[:, 8:16], in_=sR)
                            nc.vector.max_index(out=idxU[:, 16 * t + 8:16 * t + 16],
                                                in_max=v8[:, 8:16], in_values=sR)

                        # ---- idxs16[q, 32k+8t+g] = idxU[16g+q, 16t+k] via
                        # one-hot fold matmul (fp16 exact for ints <= 2048);
                        # output replicated across the 8 Q7 16-partition blocks ----
                        idxUh = sb.tile([128, 64], F16, tag="idxUh")
                        nc.vector.tensor_copy(out=idxUh, in_=idxU)
                        rhsm = sb.tile([128, 16, 4, 8], F16, tag="rhsm")
                        nc.vector.tensor_mul(
                            rhsm,
                            idxUh.rearrange("p (t k) -> p k t", t=4, k=16)
                                 .to_broadcast([128, 16, 4, 8]),
                            maskh_sb.rearrange("p (k t g) -> p k t g",
                                               k=16, t=4, g=8))
                        idx_ps = ps.tile([128, 512], F32, tag="idx_ps", bufs=1)
                        nc.tensor.matmul(idx_ps, p8h_sb,
                                         rhsm.rearrange("p k t g -> p (k t g)"),
                                         start=True, stop=True)
                        idxs16 = sb.tile([128, 512], I16, tag="idxs16")
                        nc.scalar.copy(out=idxs16, in_=idx_ps)

                        # ---- fused u|v point-major; u rows to DRAM (pad OP) ----
                        uv_ps = ps.tile([128, 4, 2 * O], F32, tag="uv", bufs=1)
                        for t in range(4):
                            nc.tensor.matmul(uv_ps[:, t, :],
                                             A[0:K_aug, 128 * t:128 * (t + 1)],
                                             wluv_sb[li], start=True, stop=True)
                        u_pm = sb.tile([128, 4, O], GDT, tag="u_pm")
                        nc.scalar.copy(out=u_pm, in_=uv_ps[:, :, 0:O])
                        v_sb = sb.tile([128, 4, O], F32, tag="v_sb", bufs=3,
                                       name="v_sb")
                        nc.scalar.copy(out=v_sb, in_=uv_ps[:, :, O:2 * O])
                        udram = dram.tile([512, OP], GDT, tag="udram", bufs=6)
                        nc.sync.dma_start(
                            udram.rearrange("(t p) o -> p t o", p=128, t=4)[:, :, 0:O],
                            u_pm)

                        # ---- batched gather: g[p, 4k+t, :] = udram[idx[128t+p, k]]
                        # (chunks of 1024 idxs; round-robin over the 4 SWDGE
                        # queues overlaps desc-gen with ring drain) ----
                        g = sbg.tile([128, 64, OP], GDT, tag="g")
                        for m in range(8):
                            nc.gpsimd.dma_gather(
                                g[:, 8 * m:8 * (m + 1), :], udram[:, :],
                                idxs16[:, 64 * m:64 * (m + 1)], 1024, 1024, OP,
                                single_packet=False, queue_num=m % NQ)

                        # ---- max over k + v + prelu + transpose to chan-major ----
                        if li < 3:
                            An = sbA.tile([LAYERS[li + 1][4], 512], F32, tag="A",
                                          name=f"A{li + 1}_{cl}")
                        else:
                            An = sbA.tile([128, 512], F32, tag="A", name=f"A4_{cl}")
                        xfm2_ps = ps.tile([O, 512], F32, tag="xfm2", bufs=1)
                        for t in range(4):
                            mx = sb.tile([128, O], F32, tag="mx")
                            nc.vector.reduce_max(
                                out=mx,
                                in_=g[:, t::4, 0:O].rearrange("p k c -> p c k"),
                                axis=AX.X, op=ALU.max)
                            mv = sb.tile([128, O], F32, tag="mv")
                            nc.vector.tensor_add(mv, mx, v_sb[:, t, :])
                            xn = sb.tile([128, O], F32, tag="xn")
                            nc.scalar.activation(xn, mv, AF.Prelu, alpha=0.2)
                            nc.tensor.transpose(xfm2_ps[:, 128 * t:128 * (t + 1)],
                                                xn, ident)
                        if li < 3:
                            nc.scalar.copy(out=An[0:O, :], in_=xfm2_ps)
                            nc.vector.memset(An[O:O + 1, :], 1.0)
                            An16 = sb.tile([O, 512], BF16, tag="An16")
                            nc.scalar.copy(out=An16, in_=xfm2_ps)
                            fd = dram.tile([O, 512], BF16, tag=f"feat{li}", bufs=8)
                            nc.sync.dma_start(fd[:, :], An16)
                            featd[cl][li] = fd
                        else:
                            nc.scalar.copy(out=An[:, :], in_=xfm2_ps)
                            An16_4 = sb.tile([128, 512], BF16, tag="An16_4")
                            nc.scalar.copy(out=An16_4, in_=xfm2_ps)
                            # ---- W5 + BN + prelu + pooling, interleaved with
                            # the remaining clouds' L4 gathers ----
                            xcat = sb.tile([128, 512], BF16, tag="w5cat")
                            nc.sync.dma_start(xcat[0:32, :], featd[cl][0][:, :])
                            nc.sync.dma_start(xcat[32:64, :], featd[cl][1][:, :])
                            nc.sync.dma_start(xcat[64:128, :], featd[cl][2][:, :])
                            for m in range(8):
                                h_ps = ps.tile([128, 512], F32, tag="h_ps",
                                               bufs=1)
                                nc.tensor.matmul(h_ps, w5g[0][:, 128 * m:128 * (m + 1)],
                                                 xcat, start=True, stop=False)
                                nc.tensor.matmul(h_ps, w5g[1][:, 128 * m:128 * (m + 1)],
                                                 An16_4, start=False, stop=True)
                                h_sb = sb.tile([128, 512], F32, tag="h_sb")
                                nc.scalar.activation(
                                    h_sb, h_ps, AF.Prelu, bias=b5[:, m:m + 1],
                                    scale=s5[:, m:m + 1], alpha=0.2,
                                    accum_out=PS[:, m, cl:cl + 1])
                                nc.vector.reduce_max(out=PM[:, m, cl:cl + 1],
                                                     in_=h_sb, axis=AX.X,
                                                     op=ALU.max)
                        A_nxt.append(An)
                    A_cur = A_nxt

            # ================= MLP over the core's 8 clouds =================
            with ExitStack() as mctx:
                msb = mctx.enter_context(tc.tile_pool(name="mlp", bufs=4))
                msb1 = mctx.enter_context(tc.tile_pool(name="mlp1", bufs=1))
                mps = mctx.enter_context(tc.tile_pool(name="mlpps", bufs=1, space="PSUM"))

                # clouds-as-lhsT orientation: out [8 clouds, channels]; BN
                # scales folded into weight columns host-side, biases via
                # ones-row accumulation matmuls.
                ones8 = msb1.tile([1, 8], F32, tag="ones8")
                nc.vector.memset(ones8, 1.0)
                b6r = msb1.tile([1, 1024], F32, tag="b6r")
                nc.sync.dma_start(b6r, din["b6row"][:, :])
                b7r = msb1.tile([1, 1024], F32, tag="b7r")
                nc.sync.dma_start(b7r, din["b7row"][:, :])
                l3br = msb1.tile([1, 40], F32, tag="l3br")
                nc.sync.dma_start(l3br, din["lin3brow"][:, :])

                z1 = msb1.tile([8, 1024], F32, tag="z1")
                for h in range(2):
                    zps = mps.tile([8, 512], F32, tag="mps", bufs=2,
                                   name=f"z1ps{h}")
                    for kk in range(16):
                        wk = msb.tile([128, 512], F32, tag="wk")
                        nc.sync.dma_start(
                            wk, din["lin1wt"][128 * kk:128 * (kk + 1),
                                              512 * h:512 * (h + 1)])
                        lhs = PM[:, kk, :] if kk < 8 else PS[:, kk - 8, :]
                        nc.tensor.matmul(zps, lhs, wk, start=(kk == 0), stop=False)
                    nc.tensor.matmul(zps, ones8, b6r[:, 512 * h:512 * (h + 1)],
                                     start=False, stop=True)
                    nc.scalar.activation(z1[:, 512 * h:512 * (h + 1)], zps,
                                         AF.Prelu, alpha=0.2)
                z1t = msb1.tile([128, 8, 8], F32, tag="z1t")
                for kk in range(8):
                    tp = mps.tile([128, 8], F32, tag="tp", bufs=2,
                                  name=f"tp{kk % 2}")
                    nc.tensor.transpose(tp, z1[:, 128 * kk:128 * (kk + 1)],
                                        ident[0:8, 0:8])
                    nc.scalar.copy(out=z1t[:, kk, :], in_=tp)
                z2 = msb1.tile([8, 1024], F32, tag="z2")
                for h in range(2):
                    zps = mps.tile([8, 512], F32, tag="mps", bufs=2,
                                   name=f"z2ps{h}")
                    for kk in range(8):
                        wk = msb.tile([128, 512], F32, tag="wk2")
                        nc.sync.dma_start(
                            wk, din["lin2wt"][128 * kk:128 * (kk + 1),
                                              512 * h:512 * (h + 1)])
                        nc.tensor.matmul(zps, z1t[:, kk, :], wk,
                                         start=(kk == 0), stop=False)
                    nc.tensor.matmul(zps, ones8, b7r[:, 512 * h:512 * (h + 1)],
                                     start=False, stop=True)
                    nc.scalar.activation(z2[:, 512 * h:512 * (h + 1)], zps,
                                         AF.Prelu, alpha=0.2)
                z2t = msb1.tile([128, 8, 8], F32, tag="z2t")
                for kk in range(8):
                    tp = mps.tile([128, 8], F32, tag="tp", bufs=2,
                                  name=f"tp{kk % 2}")
                    nc.tensor.transpose(tp, z2[:, 128 * kk:128 * (kk + 1)],
                                        ident[0:8, 0:8])
                    nc.scalar.copy(out=z2t[:, kk, :], in_=tp)
                z3_ps = mps.tile([8, 40], F32, tag="mps", bufs=2, name="z3ps")
                for kk in range(8):
                    nc.tensor.matmul(z3_ps, z2t[:, kk, :], lin3w[kk],
                                     start=(kk == 0), stop=False)
                nc.tensor.matmul(z3_ps, ones8, l3br, start=False, stop=True)
                z3 = msb1.tile([8, CLS], F32, tag="z3")
                nc.scalar.copy(out=z3, in_=z3_ps)
                nc.sync.dma_start(cc_in[:, :], z3)

                nc.gpsimd.collective_compute(
                    "AllGather", ALU.bypass,
                    replica_groups=[list(range(NCORES))],
                    ins=[cc_in[:, :]], outs=[cc_out[:, :]],
                )

            # ================= stage 2: voxel graph (replicated) =============
            with ExitStack() as s2ctx:
                tsb = s2ctx.enter_context(tc.tile_pool(name="s2", bufs=2))
                tsb1 = s2ctx.enter_context(tc.tile_pool(name="s2a", bufs=1))
                tps = s2ctx.enter_context(tc.tile_pool(name="s2ps", bufs=2, space="PSUM"))

                w6l_sb = tsb1.tile([40, 512], F32, tag="w6l")
                nc.sync.dma_start(w6l_sb, din["w6l"][:, :])
                w6d_sb = tsb1.tile([40, 512], F32, tag="w6d")
                nc.sync.dma_start(w6d_sb, din["w6d"][:, :])
                w7l_sb = [tsb1.tile([128, 256], F32, tag=f"w7l{i}", name=f"w7l{i}") for i in range(4)]
                w7d_sb = [tsb1.tile([128, 256], F32, tag=f"w7d{i}", name=f"w7d{i}") for i in range(4)]
                for i in range(4):
                    nc.sync.dma_start(w7l_sb[i], din["w7l"][128 * i:128 * (i + 1), :])
                    nc.sync.dma_start(w7d_sb[i], din["w7d"][128 * i:128 * (i + 1), :])
                w8_sb = [tsb1.tile([128, 1024], F32, tag=f"w8_{i}", name=f"w8_{i}") for i in range(6)]
                for i in range(6):
                    nc.sync.dma_start(w8_sb[i], din["w8t"][128 * i:128 * (i + 1), :])
                lin4_sb = [tsb1.tile([128, 256], F32, tag=f"l4_{i}", name=f"l4_{i}") for i in range(16)]
                for i in range(16):
                    nc.sync.dma_start(lin4_sb[i], din["lin4wt"][128 * i:128 * (i + 1), :])
                lin5_sb = [tsb1.tile([128, 40], F32, tag=f"l5_{i}", name=f"l5_{i}") for i in range(2)]
                for i in range(2):
                    nc.sync.dma_start(lin5_sb[i], din["lin5wt"][128 * i:128 * (i + 1), :])

                def knn_bias(s_ps, ctx_tag):
                    """s_ps (32, 64) psum (b0|b1 cols) -> biasRS (128, 2048)
                    SBUF, layout (b n m): -1e30 non-neighbor bias, 0 for
                    top-VK neighbors, replicated across partitions."""
                    s_sb = tsb.tile([32, 64], F32, tag=f"ssb{ctx_tag}")
                    nc.scalar.copy(out=s_sb, in_=s_ps)
                    v8 = tsb.tile([32, 16], F32, tag=f"v8{ctx_tag}")
                    nc.vector.max(out=v8[:, 0:8], in_=s_sb[:, 0:32])
                    nc.vector.max(out=v8[:, 8:16], in_=s_sb[:, 32:64])
                    bias2 = tsb.tile([32, 64], F32, tag=f"b2{ctx_tag}")
                    nc.vector.tensor_scalar(bias2[:, 0:32], s_sb[:, 0:32],
                                            v8[:, 7:8], -1.0e30,
                                            op0=ALU.is_lt, op1=ALU.mult)
                    nc.vector.tensor_scalar(bias2[:, 32:64], s_sb[:, 32:64],
                                            v8[:, 15:16], -1.0e30,
                                            op0=ALU.is_lt, op1=ALU.mult)
                    biasF = tsb.tile([1, 2048], F32, tag=f"bf{ctx_tag}")
                    for bb in range(2):
                        bdb = dram.tile([32, 32], F32, tag=f"bd{bb}")
                        nc.sync.dma_start(bdb, bias2[:, 32 * bb:32 * (bb + 1)])
                        nc.sync.dma_start(
                            biasF[:, 1024 * bb:1024 * (bb + 1)],
                            bdb.rearrange("a b -> (a b)"))
                    br_ps = tps.tile([128, 2048], F32, tag="br_ps", bufs=1)
                    for h in range(4):
                        nc.tensor.matmul(br_ps[:, 512 * h:512 * (h + 1)],
                                         ones_row[0:1, 0:128],
                                         biasF[:, 512 * h:512 * (h + 1)],
                                         start=True, stop=True)
                    biasRS = tsb.tile([128, 2048], F32, tag=f"brs{ctx_tag}")
                    nc.scalar.copy(out=biasRS, in_=br_ps)
                    return biasRS

                def edge_max(u2_ps, v2_ps, biasRS, out, bn_b, bn_s):
                    """out[p, (b m)] = prelu(bn(max_n(u2 + bias) + v2))."""
                    dense = tsb.tile([128, 2, 32, 32], F32, tag="dense")
                    nc.vector.tensor_add(
                        dense,
                        u2_ps.rearrange("p (b a m) -> p b a m", b=2, a=1, m=32)
                             .to_broadcast([128, 2, 32, 32]),
                        biasRS.rearrange("p (b n m) -> p b n m", b=2, n=32, m=32))
                    mx2 = tsb.tile([128, 2, 32], F32, tag="mx2")
                    nc.vector.reduce_max(out=mx2, in_=dense, axis=AX.X)
                    mv2 = tsb.tile([128, 64], F32, tag="mv2")
                    nc.vector.tensor_add(
                        mv2, mx2.rearrange("p b m -> p (b m)"), v2_ps)
                    nc.scalar.activation(out, mv2, AF.Prelu,
                                         bias=bn_b, scale=bn_s, alpha=0.2)

                # x (40, 64) from all 64 cc_out rows; cols 0:32 = b0, 32:64 = b1
                y0 = tsb.tile([64, 40], F32, tag="y0")
                nc.sync.dma_start(y0, cc_out[:, :])
                A2_ps = tps.tile([40, 64], F32, tag="tpsA", bufs=1)
                nc.tensor.transpose(A2_ps, y0, ident[0:64, 0:64])
                A2 = tsb.tile([40, 64], F32, tag="A2")
                nc.scalar.copy(out=A2, in_=A2_ps)

                # KNN key for stage2 layer 1 (batched norms, per-b scores)
                sq2 = tsb.tile([40, 64], F32, tag="sq2")
                nc.vector.tensor_mul(sq2, A2, A2)
                yy_ps = tps.tile([1, 64], F32, tag="tpsA", bufs=1)
                nc.tensor.matmul(yy_ps, ones_col[0:40, :], sq2,
                                 start=True, stop=True)
                nyy2 = tsb.tile([1, 64], F32, tag="nyy2")
                nc.scalar.activation(nyy2, yy_ps, AF.Identity, scale=-1.0)
                B2 = tsb.tile([40, 64], F32, tag="B2")
                nc.scalar.activation(B2, A2, AF.Identity, scale=2.0)
                s2_ps = tps.tile([32, 64], F32, tag="tpsA", bufs=1)
                for b in range(2):
                    sl = slice(32 * b, 32 * b + 32)
                    nc.tensor.matmul(s2_ps[:, sl], A2[:, sl], B2[:, sl],
                                     start=True, stop=False)
                    nc.tensor.matmul(s2_ps[:, sl], ones_row[0:1, 0:32],
                                     nyy2[:, sl], start=False, stop=True)
                biasRS = knn_bias(s2_ps, "a")

                # W6 edge layer: O=512, C=40 (both b per matmul)
                y1 = [tsb.tile([128, 64], F32, tag=f"y1_{ot}", name=f"y1_{ot}")
                      for ot in range(4)]
                for ot in range(4):
                    u2_ps = tps.tile([128, 64], F32, tag="u2ps", bufs=1)
                    v2_ps = tps.tile([128, 64], F32, tag="v2ps", bufs=1)
                    nc.tensor.matmul(u2_ps, w6l_sb[:, 128 * ot:128 * (ot + 1)],
                                     A2, start=True, stop=True)
                    nc.tensor.matmul(v2_ps, w6d_sb[:, 128 * ot:128 * (ot + 1)],
                                     A2, start=True, stop=True)
                    edge_max(u2_ps, v2_ps, biasRS, y1[ot],
                             b6c[:, ot:ot + 1], s6c[:, ot:ot + 1])

                # KNN key for stage2 layer 2 (C=512)
                yy2_ps = tps.tile([1, 64], F32, tag="tpsA", bufs=1)
                for i in range(4):
                    sq2b = tsb.tile([128, 64], F32, tag="sq2b")
                    nc.vector.tensor_mul(sq2b, y1[i], y1[i])
                    nc.tensor.matmul(yy2_ps, ones_col[:, :], sq2b,
                                     start=(i == 0), stop=(i == 3))
                nyy = tsb.tile([1, 64], F32, tag="nyy")
                nc.scalar.activation(nyy, yy2_ps, AF.Identity, scale=-1.0)
                y1x2 = [tsb.tile([128, 64], F32, tag=f"y1x2_{i}",
                                 name=f"y1x2_{i}") for i in range(4)]
                for i in range(4):
                    nc.scalar.activation(y1x2[i], y1[i], AF.Identity, scale=2.0)
                s2b_ps = tps.tile([32, 64], F32, tag="tpsA", bufs=1)
                for b in range(2):
                    sl = slice(32 * b, 32 * b + 32)
                    for i in range(4):
                        nc.tensor.matmul(s2b_ps[:, sl], y1[i][:, sl],
                                         y1x2[i][:, sl],
                                         start=(i == 0), stop=False)
                    nc.tensor.matmul(s2b_ps[:, sl], ones_row[0:1, 0:32],
                                     nyy[:, sl], start=False, stop=True)
                biasRS2 = knn_bias(s2b_ps, "b")

                # W7 edge layer: O=256, C=512
                y2 = [tsb.tile([128, 64], F32, tag=f"y2_{ot}", name=f"y2_{ot}")
                      for ot in range(2)]
                for ot in range(2):
                    u2_ps = tps.tile([128, 64], F32, tag="u2ps", bufs=1)
                    v2_ps = tps.tile([128, 64], F32, tag="v2ps", bufs=1)
                    for kk in range(4):
                        nc.tensor.matmul(u2_ps,
                                         w7l_sb[kk][:, 128 * ot:128 * (ot + 1)],
                                         y1[kk], start=(kk == 0), stop=(kk == 3))
                        nc.tensor.matmul(v2_ps,
                                         w7d_sb[kk][:, 128 * ot:128 * (ot + 1)],
                                         y1[kk], start=(kk == 0), stop=(kk == 3))
                    edge_max(u2_ps, v2_ps, biasRS2, y2[ot],
                             b7c[:, ot:ot + 1], s7c[:, ot:ot + 1])

                # W8 + BN + prelu + pooling (per-b accum/max from batched psum)
                yall = y1 + y2
                for m in range(8):
                    h2_ps = tps.tile([128, 64], F32, tag="tpsB", bufs=1)
                    for gi in range(6):
                        nc.tensor.matmul(h2_ps,
                                         w8_sb[gi][:, 128 * m:128 * (m + 1)],
                                         yall[gi], start=(gi == 0), stop=(gi == 5))
                    h2 = tsb.tile([128, 64], F32, tag="h2")
                    for b in range(2):
                        sl = slice(32 * b, 32 * b + 32)
                        nc.scalar.activation(h2[:, sl], h2_ps[:, sl], AF.Prelu,
                                             bias=b8c[:, m:m + 1],
                                             scale=s8c[:, m:m + 1],
                                             alpha=0.2,
                                             accum_out=PS2[:, m, b:b + 1])
                        nc.vector.reduce_max(out=PM2[:, m, b:b + 1],
                                             in_=h2[:, sl],
                                             axis=AX.X, op=ALU.max)

                # ---- lin4 + lin5 tail over both b's ----
                z4 = tsb1.tile([128, 2, 2], F32, tag="z4")
                for m in range(2):
                    z4_ps = tps.tile([128, 2], F32, tag="tpsB", bufs=1)
                    for kk in range(16):
                        rhs = PM2[:, kk, :] if kk < 8 else PS2[:, kk - 8, :]
                        nc.tensor.matmul(z4_ps, lin4_sb[kk][:, 128 * m:128 * (m + 1)],
                                         rhs, start=(kk == 0), stop=(kk == 15))
                    nc.scalar.activation(z4[:, m, :], z4_ps, AF.Prelu, alpha=0.2)
                z5_ps = tps.tile([40, 2], F32, tag="tpsB", bufs=1)
                for kk in range(2):
                    nc.tensor.matmul(z5_ps, lin5_sb[kk], z4[:, kk, :],
                                     start=(kk == 0), stop=(kk == 1))
                z5 = tsb1.tile([40, 2], F32, tag="z5")
                nc.scalar.activation(z5, z5_ps, AF.Identity, bias=lin5b[:, :])
                nc.sync.dma_start(out_d.rearrange("b c -> c b"), z5)

    nc.compile()
    return nc


def kernel(**inputs):
    if "nc" not in _CACHE:
        _CACHE["nc"] = _build()
    nc = _CACHE["nc"]
    host = _prep_host(inputs)
    inp = np.asarray(inputs["input"], np.float32)          # (64, 512, 9)
    in_maps = []
    for c in range(NCORES):
        m = {"inp": np.ascontiguousarray(
            inp[c * CLOUDS_PER_CORE:(c + 1) * CLOUDS_PER_CORE])}
        for k, v in host.items():
            m[k] = v
        in_maps.append(m)
    res = bass_utils.run_bass_kernel_spmd(nc, in_maps, core_ids=list(range(NCORES)))
    return np.asarray(res.results[0]["out"], np.float32)

